# revision 12
# baseline (speedup 1.0000x reference)
"""Multi-head causal self-attention on 8 Trainium2 NeuronCores.

Problem: B=4, T=2048, D=1024, H=16 heads, Hd=64. fp32 in/out.
Sharding: core c handles batch b = c//2 and head-group g = c%2 (8 heads,
512 channels). Each core computes a partial output (its head-group's
contribution to x @ Wo); the host sums head-group pairs and adds bo.

v3 (vs v1 baseline at ~365us):
  - x is transposed on the HOST; the kernel loads x^T [D, T] with plain
    DMAs split across both HWDGE queues (sync + scalar), weights first.
  - Fully interleaved emission: per q-span rounds that mix Q/K
    projection chains, V' chains, attention (ScalarE-bound exp), and the
    out-projection for the previous span, so TensorE fills exp-bound
    gaps and ScalarE starts ~10us into the kernel.
  - ctx'/denominator PSUM is evacuated to SBUF with one copy right
    after the last ctx matmul, so the single cs PSUM buffer frees
    immediately and the softmax-normalize chain runs off-critical-path.
  - Softmax norm without the DRAM bounce: denominator row -> 4KB
    SBUF->SBUF shift DMA to partition 0 -> gpsimd partition_broadcast
    -> fast reciprocal -> two multiplies (head A writes ctxT in place,
    head B shuffles to partitions 64:128 via one SBUF->SBUF DMA).

Per-core algorithm:
  x^T  [D=1024, T]   loaded directly (host transpose)
  Q^T  [C=512, T]    = matmul(lhsT=Wq chunk, rhs=x^T)   (head h at partitions
  K^T  [C=512, T]      64*(h%2) .. of chunk h//2)
  V'   [T, 8*65]     = matmul(lhsT=x^T chunk, rhs=Wv), per head [V(64) | 1]
  S^T  [k,q]         = matmul(lhsT=K^T block, rhs=Q^T span)  (k on partitions,
                       the two heads of a pair run as concurrent row-tiles)
  E = exp((S^T)/8)   on ScalarE, PSUM->SBUF; diag blocks get a 0/1 mask mult
  ctx' [65, q]       = matmul(lhsT=V' block, rhs=E)  accumulated over k blocks
                       row 64 = softmax denominator (ones-column trick)
  out  [T, D]        = matmul(lhsT=ctx^T chunk, rhs=Wo chunk), DMA out
Causality: only k-blocks with k0 <= q_span_end are computed; the <=4
diagonal blocks per span get a multiplicative staircase mask.
"""

import sys

for _p in ("/opt/trn_rl_repo", "/root/.axon_site/_ro/trn_rl_repo"):
    if _p not in sys.path:
        sys.path.append(_p)

import numpy as np

import concourse.bacc as bacc
import concourse.mybir as mybir
import concourse.tile as tile
from concourse.bass_utils import run_bass_kernel_spmd

FP32 = mybir.dt.float32
BF16 = mybir.dt.bfloat16
P = 128
T = 2048  # sequence length
D = 1024  # model dim
C = 512   # channels per core (8 heads)
H = 8     # heads per core
HD = 64   # head dim
N_CORES = 8
NSPAN = 4          # q spans of 512
SPAN = 512
NKB = 16           # k blocks of 128

_program = None


def _build(debug=False):
    nc = bacc.Bacc()
    # x^T (host-transposed): [D, T]
    xt_d = nc.declare_dram_parameter("xt", [D, T], BF16, isOutput=False)
    wq_d = nc.declare_dram_parameter("wq", [D, C], BF16, isOutput=False)
    wk_d = nc.declare_dram_parameter("wk", [D, C], BF16, isOutput=False)
    wv_d = nc.declare_dram_parameter("wv", [D, C], BF16, isOutput=False)
    wo_d = nc.declare_dram_parameter("wo", [C, D], BF16, isOutput=False)
    mask_d = nc.declare_dram_parameter("mask", [P, 1024], BF16, isOutput=False)
    out_d = nc.declare_dram_parameter("out", [T, D], FP32, isOutput=True)
    if debug:
        qt_dump = nc.declare_dram_parameter("qt_dump", [C, T], BF16, isOutput=True)
        kt_dump = nc.declare_dram_parameter("kt_dump", [C, T], BF16, isOutput=True)
        vp_dump = nc.declare_dram_parameter("vp_dump", [T, H * 65], BF16, isOutput=True)
        ct_dump = nc.declare_dram_parameter("ct_dump", [C, T], BF16, isOutput=True)

    Exp = mybir.ActivationFunctionType.Exp

    from contextlib import ExitStack

    with tile.TileContext(nc) as tc, ExitStack() as persist:
        const_pool = persist.enter_context(tc.tile_pool(name="const", bufs=1))
        qkt_pool = persist.enter_context(tc.tile_pool(name="qkt", bufs=1))
        vp_pool = persist.enter_context(tc.tile_pool(name="vp", bufs=1))
        persist_w = persist.enter_context(tc.tile_pool(name="pw", bufs=1))
        ctxT_pool = persist.enter_context(tc.tile_pool(name="ctxT", bufs=1))
        xt_pool = persist.enter_context(tc.tile_pool(name="xt", bufs=1))
        # PSUM: bps (projection/out-proj chains) 2x1 + st 2x2 + cs 1x2 = 8
        bps_pool = persist.enter_context(tc.tile_pool(name="bps", bufs=2, space="PSUM"))
        st_pool = persist.enter_context(tc.tile_pool(name="stps", bufs=2, space="PSUM"))
        cs_pool = persist.enter_context(tc.tile_pool(name="csps", bufs=1, space="PSUM"))
        e_pool = persist.enter_context(tc.tile_pool(name="epool", bufs=8))
        n_pool = persist.enter_context(tc.tile_pool(name="npool", bufs=2))
        o_pool = persist.enter_context(tc.tile_pool(name="osb", bufs=3))

        # ---- DMA loads: weights first, x^T chunks split over both HWDGE
        # queues (sync + scalar), t-half 0 before t-half 1 so V' and the
        # first q-spans unblock early.
        wq_sb = persist_w.tile([P, 8, C], BF16, tag="wq")
        wk_sb = persist_w.tile([P, 8, C], BF16, tag="wk")
        wv_sb = persist_w.tile([P, 8, C], BF16, tag="wv")
        wo_sb = persist_w.tile([P, 4, D], BF16, tag="wo")
        mask_sb = const_pool.tile([P, 1024], BF16, tag="mask")

        ones_sb = const_pool.tile([P, 64], FP32, tag="ones")
        nc.gpsimd.memset(ones_sb[:], 1.0)

        nc.sync.dma_start(wq_sb[:], wq_d.rearrange("(o p) c -> p o c", p=P))
        nc.scalar.dma_start(wk_sb[:], wk_d.rearrange("(o p) c -> p o c", p=P))
        # third DMA path: SWDGE (gpsimd) for the tensors the first
        # projection chains don't need, so sync/scalar only carry wq/wk+x^T
        nc.gpsimd.dma_start(wv_sb[:], wv_d.rearrange("(o p) c -> p o c", p=P))
        nc.gpsimd.dma_start(mask_sb[:], mask_d[:])
        nc.gpsimd.dma_start(wo_sb[:], wo_d.rearrange("(o p) d -> p o d", p=P))

        xt = [xt_pool.tile([P, T], BF16, tag=f"xt{j}", name=f"xt{j}") for j in range(8)]
        for th in range(2):
            for j in range(8):
                tsl = slice(th * (T // 2), (th + 1) * (T // 2))
                eng = nc.sync if j % 2 == 0 else nc.scalar
                eng.dma_start(xt[j][:, tsl], xt_d[j * P:(j + 1) * P, tsl])

        qt = [qkt_pool.tile([P, T], BF16, tag=f"qt{i}", name=f"qt{i}") for i in range(4)]
        kt = [qkt_pool.tile([P, T], BF16, tag=f"kt{i}", name=f"kt{i}") for i in range(4)]
        vp = [vp_pool.tile([P, H, 65], BF16, tag=f"vp{t}", name=f"vp{t}") for t in range(NKB)]
        ctxT = [ctxT_pool.tile([P, T], BF16, tag=f"ct{i}", name=f"ct{i}")
                for i in range(4)]

        ci = 0

        def copy_px(dst, src, allow_scalar):
            # PSUM->SBUF evacuation; ScalarE only while it is not yet
            # exp-saturated (early rounds), VectorE otherwise.
            nonlocal ci
            ci += 1
            if allow_scalar and ci % 2 == 0:
                nc.scalar.copy(dst, src)
            else:
                nc.vector.tensor_copy(dst, src)

        def emit_qk(hp, s, allow_scalar=False):
            for dst, wsb in ((kt, wk_sb), (qt, wq_sb)):
                ps = bps_pool.tile([P, SPAN], FP32, tag="bps")
                for j in range(8):
                    nc.tensor.matmul(
                        ps[:],
                        wsb[:, j, hp * P:(hp + 1) * P],
                        xt[j][:, s * SPAN:(s + 1) * SPAN],
                        start=(j == 0), stop=(j == 7),
                    )
                copy_px(dst[hp][:, s * SPAN:(s + 1) * SPAN], ps[:], allow_scalar)

        def emit_vp(t, allow_scalar=False):
            nc.gpsimd.memset(vp[t][:], 1.0)
            ps = bps_pool.tile([P, C], FP32, tag="bps")
            for j in range(8):
                nc.tensor.matmul(
                    ps[:],
                    xt[j][:, t * P:(t + 1) * P],
                    wv_sb[:, j, :],
                    start=(j == 0), stop=(j == 7),
                )
            copy_px(vp[t][:, :, 0:64],
                    ps.rearrange("p (h e) -> p h e", e=64), allow_scalar)

        def emit_C(s, hp):
            hA, hB = 2 * hp, 2 * hp + 1
            # cs: [65, 0:512] = head A ctx' (row 64 denom), [.., 512:] = B
            cs = cs_pool.tile([P, 1024], FP32, tag="cs")
            nkb = 4 * s + 4
            for kb in range(nkb):
                ksl = slice(kb * P, (kb + 1) * P)
                d = max(0, kb - 4 * s)      # diagonal offset 0..3
                q0 = s * SPAN + 128 * d     # valid q start
                w = SPAN - 128 * d          # valid width
                qsl = slice(q0, (s + 1) * SPAN)
                st = st_pool.tile([P, 1024], FP32, tag="st")
                st3 = st.rearrange("p (b q) -> p b q", b=2)[:, :, 0:w]
                nc.tensor.matmul(st[:, 0:w], kt[hp][0:64, ksl],
                                 qt[hp][0:64, qsl],
                                 start=True, stop=True)
                nc.tensor.matmul(st[:, 512:512 + w], kt[hp][64:128, ksl],
                                 qt[hp][64:128, qsl],
                                 start=True, stop=True)
                e = e_pool.tile([P, 1024], BF16, tag="e")
                e3 = e.rearrange("p (b q) -> p b q", b=2)[:, :, 0:w]
                nc.scalar.activation(e3, st3, Exp, scale=0.125)
                if d > 0 or kb == 4 * s:
                    # the staircase only affects the first 128 columns of
                    # the valid window (q0 == k0 for diagonal blocks)
                    e3m = e.rearrange("p (b q) -> p b q", b=2)[:, :, 0:128]
                    m3 = mask_sb[:, None, 384:512]
                    nc.vector.tensor_mul(
                        e3m, e3m, m3.to_broadcast((P, 2, 128)))
                co = 128 * d
                nc.tensor.matmul(cs[0:65, co:SPAN],
                                 vp[kb][:, hA, :],
                                 e[:, 0:w],
                                 start=(kb == 0), stop=(kb == nkb - 1))
                nc.tensor.matmul(cs[0:65, 512 + co:1024],
                                 vp[kb][:, hB, :],
                                 e[:, 512:512 + w],
                                 start=(kb == 0), stop=(kb == nkb - 1))
            qsl = slice(s * SPAN, (s + 1) * SPAN)
            rrb = n_pool.tile([P, 1024], FP32, tag="rrb")
            tmp = n_pool.tile([P, SPAN], BF16, tag="tmp")
            if s == 3 and hp == 3:
                # Tail: low-latency normalize. Denominator row -> SBUF,
                # broadcast to partitions 0..63 with a rank-1 matmul
                # (ones outer product; PE is idle here), reciprocal from
                # PSUM at base 0, multiply straight out of cs.
                rs = n_pool.tile([P, 1024], FP32, tag="csb")
                nc.vector.tensor_copy(rs[64:65, :], cs[64:65, :])
                psb = st_pool.tile([P, 1024], FP32, tag="st")
                for half in range(2):
                    hsl = slice(half * 512, (half + 1) * 512)
                    nc.tensor.matmul(psb[0:64, hsl], ones_sb[64:65, :],
                                     rs[64:65, hsl], start=True, stop=True)
                nc.vector.reciprocal_approx_fast(rrb[0:64, :], psb[0:64, :])
                nc.vector.tensor_mul(ctxT[hp][0:64, qsl],
                                     cs[0:64, 0:512], rrb[0:64, 0:512])
                nc.vector.tensor_mul(tmp[0:64, :],
                                     cs[0:64, 512:1024], rrb[0:64, 512:1024])
            else:
                # Evacuate ctx'+denominators to SBUF (frees the single cs
                # PSUM buffer), then normalize rows 0..63 by row 64:
                # shift-DMA the denom row to partition 0, gpsimd
                # partition_broadcast, fast reciprocal, multiply. Head A
                # lands in ctxT directly; head B shuffles to partitions
                # 64:128 by SBUF->SBUF DMA.
                csb = n_pool.tile([P, 1024], FP32, tag="csb")
                r0 = n_pool.tile([1, 1024], FP32, tag="r0")
                rsb = n_pool.tile([P, 1024], FP32, tag="rsb")
                nc.vector.tensor_copy(csb[0:65, :], cs[0:65, :])
                nc.sync.dma_start(r0[0:1, :], csb[64:65, :])
                nc.gpsimd.partition_broadcast(rsb[0:64, :], r0[0:1, :])
                nc.vector.reciprocal_approx_fast(rrb[0:64, :], rsb[0:64, :])
                nc.vector.tensor_mul(ctxT[hp][0:64, qsl],
                                     csb[0:64, 0:512], rrb[0:64, 0:512])
                nc.vector.tensor_mul(tmp[0:64, :],
                                     csb[0:64, 512:1024], rrb[0:64, 512:1024])
            nc.sync.dma_start(ctxT[hp][64:128, qsl], tmp[0:64, :])

        def emit_D_qb(qb):
            pss = [bps_pool.tile([P, SPAN], FP32, tag="bps", name=f"ops{qb}_{nh}")
                   for nh in range(2)]
            for hp in range(4):
                for nh in range(2):
                    nc.tensor.matmul(
                        pss[nh][:],
                        ctxT[hp][:, qb * P:(qb + 1) * P],
                        wo_sb[:, hp, nh * SPAN:(nh + 1) * SPAN],
                        start=(hp == 0), stop=(hp == 3),
                    )
            for nh in range(2):
                ot = o_pool.tile([P, SPAN], FP32, tag="osb")
                nc.vector.tensor_copy(ot[:], pss[nh][:])
                nc.sync.dma_start(
                    out_d[qb * P:(qb + 1) * P, nh * SPAN:(nh + 1) * SPAN],
                    ot[:])

        # ---- Interleaved rounds: one per q-span. V'/QK chains for the
        # NEXT round are prefetched inside the current round so each
        # round opens directly with attention (ScalarE never waits on a
        # projection chain), and the previous span's out-projection is
        # spread through the round to fill TensorE under exp.
        emit_qk(0, 0, allow_scalar=True)
        for t in range(4):
            emit_vp(t, allow_scalar=True)
        for s in range(NSPAN):
            early = s < 2
            emit_C(s, 0)
            for hp in range(1, 4):
                emit_qk(hp, s, allow_scalar=early)
            emit_C(s, 1)
            if s > 0:
                emit_D_qb(4 * (s - 1) + 0)
            if s < 3:
                emit_vp(4 * (s + 1) + 0, allow_scalar=early)
                emit_vp(4 * (s + 1) + 1, allow_scalar=early)
            emit_C(s, 2)
            if s > 0:
                emit_D_qb(4 * (s - 1) + 1)
            if s < 3:
                emit_vp(4 * (s + 1) + 2, allow_scalar=early)
                emit_vp(4 * (s + 1) + 3, allow_scalar=early)
            emit_C(s, 3)
            if s > 0:
                emit_D_qb(4 * (s - 1) + 2)
                emit_D_qb(4 * (s - 1) + 3)
            if s < 3:
                emit_qk(0, s + 1, allow_scalar=early)
        for qb in range(12, 16):
            emit_D_qb(qb)

        if debug:
            for t in range(NKB):
                nc.sync.dma_start(vp_dump[t * P:(t + 1) * P, :],
                                  vp[t].rearrange("p h e -> p (h e)"))
            for i in range(4):
                nc.sync.dma_start(qt_dump[i * P:(i + 1) * P, :], qt[i][:])
                nc.sync.dma_start(kt_dump[i * P:(i + 1) * P, :], kt[i][:])
                nc.sync.dma_start(ct_dump[i * P:(i + 1) * P, :], ctxT[i][:])

    nc.compile()
    return nc


def _get_program():
    global _program
    if _program is None:
        _program = _build()
    return _program


def _make_mask():
    import ml_dtypes
    j = np.arange(1024)[None, :]
    k = np.arange(P)[:, None]
    return np.where(j >= k + 384, 1.0, 0.0).astype(ml_dtypes.bfloat16)


def _make_in_maps(x, Wq, Wk, Wv, Wo):
    import ml_dtypes
    bf16 = ml_dtypes.bfloat16
    mask = _make_mask()
    x = np.asarray(x, np.float32)
    xts = [np.ascontiguousarray(x[b].T.astype(bf16)) for b in range(x.shape[0])]
    in_maps = []
    for c in range(N_CORES):
        b, g = c // 2, c % 2
        cols = slice(g * C, (g + 1) * C)
        in_maps.append({
            "xt": xts[b],
            "wq": np.ascontiguousarray(np.asarray(Wq[:, cols], np.float32).astype(bf16)),
            "wk": np.ascontiguousarray(np.asarray(Wk[:, cols], np.float32).astype(bf16)),
            "wv": np.ascontiguousarray(np.asarray(Wv[:, cols], np.float32).astype(bf16)),
            "wo": np.ascontiguousarray(np.asarray(Wo[cols, :], np.float32).astype(bf16)),
            "mask": mask,
        })
    return in_maps


def _combine(results, bo, B):
    out = np.empty((B, T, D), dtype=np.float32)
    bo = np.asarray(bo, dtype=np.float32)
    for b in range(B):
        out[b] = results[2 * b]["out"] + results[2 * b + 1]["out"] + bo
    return out


def kernel(x, Wq, Wk, Wv, Wo, bo):
    x = np.asarray(x)
    nc = _get_program()
    in_maps = _make_in_maps(x, Wq, Wk, Wv, Wo)
    res = run_bass_kernel_spmd(nc, in_maps, core_ids=list(range(N_CORES)))
    return _combine(res.results, bo, x.shape[0])


def kernel_traced(x, Wq, Wk, Wv, Wo, bo):
    """Like kernel() but also returns the BassKernelResults (with
    exec_time_ns when NTFF tracing is available)."""
    x = np.asarray(x)
    nc = _get_program()
    in_maps = _make_in_maps(x, Wq, Wk, Wv, Wo)
    res = run_bass_kernel_spmd(nc, in_maps, core_ids=list(range(N_CORES)),
                               trace=True)
    return _combine(res.results, bo, x.shape[0]), res


# revision 17
# speedup vs baseline: 1.0095x; 1.0095x over previous
"""Multi-head causal self-attention on 8 Trainium2 NeuronCores.

Problem: B=4, T=2048, D=1024, H=16 heads, Hd=64. fp32 in/out.
Sharding: core c handles batch b = c//2 and head-group g = c%2 (8 heads,
512 channels). Each core computes a partial output (its head-group's
contribution to x @ Wo); the host sums head-group pairs and adds bo.

v3 (vs v1 baseline at ~365us):
  - x is transposed on the HOST; the kernel loads x^T [D, T] with plain
    DMAs split across both HWDGE queues (sync + scalar), weights first.
  - Fully interleaved emission: per q-span rounds that mix Q/K
    projection chains, V' chains, attention (ScalarE-bound exp), and the
    out-projection for the previous span, so TensorE fills exp-bound
    gaps and ScalarE starts ~10us into the kernel.
  - ctx'/denominator PSUM is evacuated to SBUF with one copy right
    after the last ctx matmul, so the single cs PSUM buffer frees
    immediately and the softmax-normalize chain runs off-critical-path.
  - Softmax norm without the DRAM bounce: denominator row -> 4KB
    SBUF->SBUF shift DMA to partition 0 -> gpsimd partition_broadcast
    -> fast reciprocal -> two multiplies (head A writes ctxT in place,
    head B shuffles to partitions 64:128 via one SBUF->SBUF DMA).

Per-core algorithm:
  x^T  [D=1024, T]   loaded directly (host transpose)
  Q^T  [C=512, T]    = matmul(lhsT=Wq chunk, rhs=x^T)   (head h at partitions
  K^T  [C=512, T]      64*(h%2) .. of chunk h//2)
  V'   [T, 8*65]     = matmul(lhsT=x^T chunk, rhs=Wv), per head [V(64) | 1]
  S^T  [k,q]         = matmul(lhsT=K^T block, rhs=Q^T span)  (k on partitions,
                       the two heads of a pair run as concurrent row-tiles)
  E = exp((S^T)/8)   on ScalarE, PSUM->SBUF; diag blocks get a 0/1 mask mult
  ctx' [65, q]       = matmul(lhsT=V' block, rhs=E)  accumulated over k blocks
                       row 64 = softmax denominator (ones-column trick)
  out  [T, D]        = matmul(lhsT=ctx^T chunk, rhs=Wo chunk), DMA out
Causality: only k-blocks with k0 <= q_span_end are computed; the <=4
diagonal blocks per span get a multiplicative staircase mask.
"""

import sys

for _p in ("/opt/trn_rl_repo", "/root/.axon_site/_ro/trn_rl_repo"):
    if _p not in sys.path:
        sys.path.append(_p)

import numpy as np

import concourse.bacc as bacc
import concourse.mybir as mybir
import concourse.tile as tile
from concourse.bass_utils import run_bass_kernel_spmd

FP32 = mybir.dt.float32
BF16 = mybir.dt.bfloat16
P = 128
T = 2048  # sequence length
D = 1024  # model dim
C = 512   # channels per core (8 heads)
H = 8     # heads per core
HD = 64   # head dim
N_CORES = 8
NSPAN = 4          # q spans of 512
SPAN = 512
NKB = 16           # k blocks of 128

_program = None


def _build(debug=False):
    nc = bacc.Bacc()
    # x^T (host-transposed): [D, T]
    xt_d = nc.declare_dram_parameter("xt", [D, T], BF16, isOutput=False)
    # weights pre-rearranged on host: [p, chunk, out] so the loads are
    # clean contiguous per-partition lines (the on-the-fly rearrange was
    # a 1KB-granular gather that ran at ~90GB/s and blocked the queue)
    wq_d = nc.declare_dram_parameter("wq", [P, 8, C], BF16, isOutput=False)
    wk_d = nc.declare_dram_parameter("wk", [P, 8, C], BF16, isOutput=False)
    wv_d = nc.declare_dram_parameter("wv", [P, 8, C], BF16, isOutput=False)
    wo_d = nc.declare_dram_parameter("wo", [P, 4, D], BF16, isOutput=False)
    mask_d = nc.declare_dram_parameter("mask", [P, 1024], BF16, isOutput=False)
    out_d = nc.declare_dram_parameter("out", [T, D], FP32, isOutput=True)
    if debug:
        qt_dump = nc.declare_dram_parameter("qt_dump", [C, T], BF16, isOutput=True)
        kt_dump = nc.declare_dram_parameter("kt_dump", [C, T], BF16, isOutput=True)
        vp_dump = nc.declare_dram_parameter("vp_dump", [T, H * 65], BF16, isOutput=True)
        ct_dump = nc.declare_dram_parameter("ct_dump", [C, T], BF16, isOutput=True)

    Exp = mybir.ActivationFunctionType.Exp

    from contextlib import ExitStack

    with tile.TileContext(nc) as tc, ExitStack() as persist:
        const_pool = persist.enter_context(tc.tile_pool(name="const", bufs=1))
        qkt_pool = persist.enter_context(tc.tile_pool(name="qkt", bufs=1))
        vp_pool = persist.enter_context(tc.tile_pool(name="vp", bufs=1))
        persist_w = persist.enter_context(tc.tile_pool(name="pw", bufs=1))
        ctxT_pool = persist.enter_context(tc.tile_pool(name="ctxT", bufs=1))
        xt_pool = persist.enter_context(tc.tile_pool(name="xt", bufs=1))
        # PSUM: bps (projection/out-proj chains) 2x1 + st 2x2 + cs 1x2 = 8
        bps_pool = persist.enter_context(tc.tile_pool(name="bps", bufs=2, space="PSUM"))
        st_pool = persist.enter_context(tc.tile_pool(name="stps", bufs=2, space="PSUM"))
        cs_pool = persist.enter_context(tc.tile_pool(name="csps", bufs=1, space="PSUM"))
        e_pool = persist.enter_context(tc.tile_pool(name="epool", bufs=8))
        n_pool = persist.enter_context(tc.tile_pool(name="npool", bufs=2))
        o_pool = persist.enter_context(tc.tile_pool(name="osb", bufs=3))

        # ---- DMA loads: weights first, x^T chunks split over both HWDGE
        # queues (sync + scalar), t-half 0 before t-half 1 so V' and the
        # first q-spans unblock early.
        wq_sb = persist_w.tile([P, 8, C], BF16, tag="wq")
        wk_sb = persist_w.tile([P, 8, C], BF16, tag="wk")
        wv_sb = persist_w.tile([P, 8, C], BF16, tag="wv")
        wo_sb = persist_w.tile([P, 4, D], BF16, tag="wo")
        mask_sb = const_pool.tile([P, 1024], BF16, tag="mask")

        ones_sb = const_pool.tile([P, 64], FP32, tag="ones")
        nc.gpsimd.memset(ones_sb[:], 1.0)

        nc.sync.dma_start(wq_sb[:], wq_d[:])
        nc.scalar.dma_start(wk_sb[:], wk_d[:])
        # third DMA path: SWDGE (gpsimd) for the tensors the first
        # projection chains don't need, so sync/scalar only carry wq/wk+x^T
        nc.gpsimd.dma_start(wv_sb[:], wv_d[:])
        nc.gpsimd.dma_start(mask_sb[:], mask_d[:])
        nc.gpsimd.dma_start(wo_sb[:], wo_d[:])

        xt = [xt_pool.tile([P, T], BF16, tag=f"xt{j}", name=f"xt{j}") for j in range(8)]
        for th in range(2):
            for j in range(8):
                tsl = slice(th * (T // 2), (th + 1) * (T // 2))
                eng = nc.sync if j % 2 == 0 else nc.scalar
                eng.dma_start(xt[j][:, tsl], xt_d[j * P:(j + 1) * P, tsl])

        qt = [qkt_pool.tile([P, T], BF16, tag=f"qt{i}", name=f"qt{i}") for i in range(4)]
        kt = [qkt_pool.tile([P, T], BF16, tag=f"kt{i}", name=f"kt{i}") for i in range(4)]
        vp = [vp_pool.tile([P, H, 65], BF16, tag=f"vp{t}", name=f"vp{t}") for t in range(NKB)]
        ctxT = [ctxT_pool.tile([P, T], BF16, tag=f"ct{i}", name=f"ct{i}")
                for i in range(4)]

        ci = 0

        def copy_px(dst, src, allow_scalar):
            # PSUM->SBUF evacuation; ScalarE only while it is not yet
            # exp-saturated (early rounds), VectorE otherwise.
            nonlocal ci
            ci += 1
            if allow_scalar and ci % 2 == 0:
                nc.scalar.copy(dst, src)
            else:
                nc.vector.tensor_copy(dst, src)

        def emit_qk(hp, s, allow_scalar=False):
            for dst, wsb in ((kt, wk_sb), (qt, wq_sb)):
                ps = bps_pool.tile([P, SPAN], FP32, tag="bps")
                for j in range(8):
                    nc.tensor.matmul(
                        ps[:],
                        wsb[:, j, hp * P:(hp + 1) * P],
                        xt[j][:, s * SPAN:(s + 1) * SPAN],
                        start=(j == 0), stop=(j == 7),
                    )
                copy_px(dst[hp][:, s * SPAN:(s + 1) * SPAN], ps[:], allow_scalar)

        def emit_vp(t, allow_scalar=False):
            # only the ones column needs the memset; V fills the rest
            nc.gpsimd.memset(vp[t][:, :, 64:65], 1.0)
            ps = bps_pool.tile([P, C], FP32, tag="bps")
            for j in range(8):
                nc.tensor.matmul(
                    ps[:],
                    xt[j][:, t * P:(t + 1) * P],
                    wv_sb[:, j, :],
                    start=(j == 0), stop=(j == 7),
                )
            copy_px(vp[t][:, :, 0:64],
                    ps.rearrange("p (h e) -> p h e", e=64), allow_scalar)

        def emit_C(s, hp):
            hA, hB = 2 * hp, 2 * hp + 1
            # cs: [65, 0:512] = head A ctx' (row 64 denom), [.., 512:] = B
            cs = cs_pool.tile([P, 1024], FP32, tag="cs")
            nkb = 4 * s + 4
            for kb in range(nkb):
                ksl = slice(kb * P, (kb + 1) * P)
                d = max(0, kb - 4 * s)      # diagonal offset 0..3
                q0 = s * SPAN + 128 * d     # valid q start
                w = SPAN - 128 * d          # valid width
                qsl = slice(q0, (s + 1) * SPAN)
                st = st_pool.tile([P, 1024], FP32, tag="st")
                st3 = st.rearrange("p (b q) -> p b q", b=2)[:, :, 0:w]
                nc.tensor.matmul(st[:, 0:w], kt[hp][0:64, ksl],
                                 qt[hp][0:64, qsl],
                                 start=True, stop=True)
                nc.tensor.matmul(st[:, 512:512 + w], kt[hp][64:128, ksl],
                                 qt[hp][64:128, qsl],
                                 start=True, stop=True)
                e = e_pool.tile([P, 1024], BF16, tag="e")
                e3 = e.rearrange("p (b q) -> p b q", b=2)[:, :, 0:w]
                nc.scalar.activation(e3, st3, Exp, scale=0.125)
                if d > 0 or kb == 4 * s:
                    # the staircase only affects the first 128 columns of
                    # the valid window (q0 == k0 for diagonal blocks)
                    e3m = e.rearrange("p (b q) -> p b q", b=2)[:, :, 0:128]
                    m3 = mask_sb[:, None, 384:512]
                    nc.vector.tensor_mul(
                        e3m, e3m, m3.to_broadcast((P, 2, 128)))
                co = 128 * d
                nc.tensor.matmul(cs[0:65, co:SPAN],
                                 vp[kb][:, hA, :],
                                 e[:, 0:w],
                                 start=(kb == 0), stop=(kb == nkb - 1))
                nc.tensor.matmul(cs[0:65, 512 + co:1024],
                                 vp[kb][:, hB, :],
                                 e[:, 512:512 + w],
                                 start=(kb == 0), stop=(kb == nkb - 1))
            qsl = slice(s * SPAN, (s + 1) * SPAN)
            rrb = n_pool.tile([P, 1024], FP32, tag="rrb")
            tmp = n_pool.tile([P, SPAN], BF16, tag="tmp")
            if s == 3 and hp == 3:
                # Tail: low-latency normalize. Denominator row -> SBUF,
                # broadcast to partitions 0..63 with a rank-1 matmul
                # (ones outer product; PE is idle here), reciprocal from
                # PSUM at base 0, multiply straight out of cs.
                rs = n_pool.tile([P, 1024], FP32, tag="csb")
                nc.vector.tensor_copy(rs[64:65, :], cs[64:65, :])
                psb = st_pool.tile([P, 1024], FP32, tag="st")
                for half in range(2):
                    hsl = slice(half * 512, (half + 1) * 512)
                    nc.tensor.matmul(psb[0:64, hsl], ones_sb[64:65, :],
                                     rs[64:65, hsl], start=True, stop=True)
                nc.vector.reciprocal_approx_fast(rrb[0:64, :], psb[0:64, :])
                nc.vector.tensor_mul(ctxT[hp][0:64, qsl],
                                     cs[0:64, 0:512], rrb[0:64, 0:512])
                nc.vector.tensor_mul(tmp[0:64, :],
                                     cs[0:64, 512:1024], rrb[0:64, 512:1024])
            else:
                # Evacuate ctx'+denominators to SBUF (frees the single cs
                # PSUM buffer), then normalize rows 0..63 by row 64:
                # shift-DMA the denom row to partition 0, gpsimd
                # partition_broadcast, fast reciprocal, multiply. Head A
                # lands in ctxT directly; head B shuffles to partitions
                # 64:128 by SBUF->SBUF DMA.
                csb = n_pool.tile([P, 1024], FP32, tag="csb")
                r0 = n_pool.tile([1, 1024], FP32, tag="r0")
                rsb = n_pool.tile([P, 1024], FP32, tag="rsb")
                nc.vector.tensor_copy(csb[0:65, :], cs[0:65, :])
                nc.sync.dma_start(r0[0:1, :], csb[64:65, :])
                nc.gpsimd.partition_broadcast(rsb[0:64, :], r0[0:1, :])
                nc.vector.reciprocal_approx_fast(rrb[0:64, :], rsb[0:64, :])
                nc.vector.tensor_mul(ctxT[hp][0:64, qsl],
                                     csb[0:64, 0:512], rrb[0:64, 0:512])
                nc.vector.tensor_mul(tmp[0:64, :],
                                     csb[0:64, 512:1024], rrb[0:64, 512:1024])
            nc.sync.dma_start(ctxT[hp][64:128, qsl], tmp[0:64, :])

        def emit_D_qb(qb):
            pss = [bps_pool.tile([P, SPAN], FP32, tag="bps", name=f"ops{qb}_{nh}")
                   for nh in range(2)]
            for hp in range(4):
                for nh in range(2):
                    nc.tensor.matmul(
                        pss[nh][:],
                        ctxT[hp][:, qb * P:(qb + 1) * P],
                        wo_sb[:, hp, nh * SPAN:(nh + 1) * SPAN],
                        start=(hp == 0), stop=(hp == 3),
                    )
            for nh in range(2):
                ot = o_pool.tile([P, SPAN], FP32, tag="osb")
                nc.vector.tensor_copy(ot[:], pss[nh][:])
                nc.sync.dma_start(
                    out_d[qb * P:(qb + 1) * P, nh * SPAN:(nh + 1) * SPAN],
                    ot[:])

        # ---- Interleaved rounds: one per q-span. V'/QK chains for the
        # NEXT round are prefetched inside the current round so each
        # round opens directly with attention (ScalarE never waits on a
        # projection chain), and the previous span's out-projection is
        # spread through the round to fill TensorE under exp.
        emit_qk(0, 0, allow_scalar=True)
        for t in range(4):
            emit_vp(t, allow_scalar=True)
        for s in range(NSPAN):
            early = s < 2
            emit_C(s, 0)
            for hp in range(1, 4):
                emit_qk(hp, s, allow_scalar=early)
            emit_C(s, 1)
            if s > 0:
                emit_D_qb(4 * (s - 1) + 0)
            if s < 3:
                emit_vp(4 * (s + 1) + 0, allow_scalar=early)
                emit_vp(4 * (s + 1) + 1, allow_scalar=early)
            emit_C(s, 2)
            if s > 0:
                emit_D_qb(4 * (s - 1) + 1)
            if s < 3:
                emit_vp(4 * (s + 1) + 2, allow_scalar=early)
                emit_vp(4 * (s + 1) + 3, allow_scalar=early)
            emit_C(s, 3)
            if s > 0:
                emit_D_qb(4 * (s - 1) + 2)
                emit_D_qb(4 * (s - 1) + 3)
            if s < 3:
                emit_qk(0, s + 1, allow_scalar=early)
        for qb in range(12, 16):
            emit_D_qb(qb)

        if debug:
            for t in range(NKB):
                nc.sync.dma_start(vp_dump[t * P:(t + 1) * P, :],
                                  vp[t].rearrange("p h e -> p (h e)"))
            for i in range(4):
                nc.sync.dma_start(qt_dump[i * P:(i + 1) * P, :], qt[i][:])
                nc.sync.dma_start(kt_dump[i * P:(i + 1) * P, :], kt[i][:])
                nc.sync.dma_start(ct_dump[i * P:(i + 1) * P, :], ctxT[i][:])

    nc.compile()
    return nc


def _get_program():
    global _program
    if _program is None:
        _program = _build()
    return _program


def _make_mask():
    import ml_dtypes
    j = np.arange(1024)[None, :]
    k = np.arange(P)[:, None]
    return np.where(j >= k + 384, 1.0, 0.0).astype(ml_dtypes.bfloat16)


def _make_in_maps(x, Wq, Wk, Wv, Wo):
    import ml_dtypes
    bf16 = ml_dtypes.bfloat16
    mask = _make_mask()
    x = np.asarray(x, np.float32)
    xts = [np.ascontiguousarray(x[b].T.astype(bf16)) for b in range(x.shape[0])]
    def chunked(w, nchunk):
        # [D_in, ncols] -> [128, nchunk, ncols]: partition p holds input
        # rows p, p+128, ... so each partition line loads contiguously
        w = np.asarray(w, np.float32).reshape(nchunk, P, -1).transpose(1, 0, 2)
        return np.ascontiguousarray(w.astype(bf16))

    in_maps = []
    for c in range(N_CORES):
        b, g = c // 2, c % 2
        cols = slice(g * C, (g + 1) * C)
        in_maps.append({
            "xt": xts[b],
            "wq": chunked(np.asarray(Wq, np.float32)[:, cols], 8),
            "wk": chunked(np.asarray(Wk, np.float32)[:, cols], 8),
            "wv": chunked(np.asarray(Wv, np.float32)[:, cols], 8),
            "wo": chunked(np.asarray(Wo, np.float32)[cols, :], 4),
            "mask": mask,
        })
    return in_maps


def _combine(results, bo, B):
    out = np.empty((B, T, D), dtype=np.float32)
    bo = np.asarray(bo, dtype=np.float32)
    for b in range(B):
        out[b] = results[2 * b]["out"] + results[2 * b + 1]["out"] + bo
    return out


def kernel(x, Wq, Wk, Wv, Wo, bo):
    x = np.asarray(x)
    nc = _get_program()
    in_maps = _make_in_maps(x, Wq, Wk, Wv, Wo)
    res = run_bass_kernel_spmd(nc, in_maps, core_ids=list(range(N_CORES)))
    return _combine(res.results, bo, x.shape[0])


def kernel_traced(x, Wq, Wk, Wv, Wo, bo):
    """Like kernel() but also returns the BassKernelResults (with
    exec_time_ns when NTFF tracing is available)."""
    x = np.asarray(x)
    nc = _get_program()
    in_maps = _make_in_maps(x, Wq, Wk, Wv, Wo)
    res = run_bass_kernel_spmd(nc, in_maps, core_ids=list(range(N_CORES)),
                               trace=True)
    return _combine(res.results, bo, x.shape[0]), res


# revision 19
# speedup vs baseline: 1.0199x; 1.0102x over previous
"""Multi-head causal self-attention on 8 Trainium2 NeuronCores.

Problem: B=4, T=2048, D=1024, H=16 heads, Hd=64. fp32 in/out.
Sharding: core c handles batch b = c//2 and head-group g = c%2 (8 heads,
512 channels). Each core computes a partial output (its head-group's
contribution to x @ Wo); the host sums head-group pairs and adds bo.

v3 (vs v1 baseline at ~365us):
  - x is transposed on the HOST; the kernel loads x^T [D, T] with plain
    DMAs split across both HWDGE queues (sync + scalar), weights first.
  - Fully interleaved emission: per q-span rounds that mix Q/K
    projection chains, V' chains, attention (ScalarE-bound exp), and the
    out-projection for the previous span, so TensorE fills exp-bound
    gaps and ScalarE starts ~10us into the kernel.
  - ctx'/denominator PSUM is evacuated to SBUF with one copy right
    after the last ctx matmul, so the single cs PSUM buffer frees
    immediately and the softmax-normalize chain runs off-critical-path.
  - Softmax norm without the DRAM bounce: denominator row -> 4KB
    SBUF->SBUF shift DMA to partition 0 -> gpsimd partition_broadcast
    -> fast reciprocal -> two multiplies (head A writes ctxT in place,
    head B shuffles to partitions 64:128 via one SBUF->SBUF DMA).

Per-core algorithm:
  x^T  [D=1024, T]   loaded directly (host transpose)
  Q^T  [C=512, T]    = matmul(lhsT=Wq chunk, rhs=x^T)   (head h at partitions
  K^T  [C=512, T]      64*(h%2) .. of chunk h//2)
  V'   [T, 8*65]     = matmul(lhsT=x^T chunk, rhs=Wv), per head [V(64) | 1]
  S^T  [k,q]         = matmul(lhsT=K^T block, rhs=Q^T span)  (k on partitions,
                       the two heads of a pair run as concurrent row-tiles)
  E = exp((S^T)/8)   on ScalarE, PSUM->SBUF; diag blocks get a 0/1 mask mult
  ctx' [65, q]       = matmul(lhsT=V' block, rhs=E)  accumulated over k blocks
                       row 64 = softmax denominator (ones-column trick)
  out  [T, D]        = matmul(lhsT=ctx^T chunk, rhs=Wo chunk), DMA out
Causality: only k-blocks with k0 <= q_span_end are computed; the <=4
diagonal blocks per span get a multiplicative staircase mask.
"""

import sys

for _p in ("/opt/trn_rl_repo", "/root/.axon_site/_ro/trn_rl_repo"):
    if _p not in sys.path:
        sys.path.append(_p)

import numpy as np

import concourse.bacc as bacc
import concourse.mybir as mybir
import concourse.tile as tile
from concourse.bass_utils import run_bass_kernel_spmd

FP32 = mybir.dt.float32
BF16 = mybir.dt.bfloat16
P = 128
T = 2048  # sequence length
D = 1024  # model dim
C = 512   # channels per core (8 heads)
H = 8     # heads per core
HD = 64   # head dim
N_CORES = 8
NSPAN = 4          # q spans of 512
SPAN = 512
NKB = 16           # k blocks of 128

_program = None


def _build(debug=False):
    nc = bacc.Bacc()
    # x^T (host-transposed): [D, T]
    xt_d = nc.declare_dram_parameter("xt", [D, T], BF16, isOutput=False)
    # weights pre-rearranged on host: [p, chunk, out] so the loads are
    # clean contiguous per-partition lines (the on-the-fly rearrange was
    # a 1KB-granular gather that ran at ~90GB/s and blocked the queue)
    wq_d = nc.declare_dram_parameter("wq", [P, 8, C], BF16, isOutput=False)
    wk_d = nc.declare_dram_parameter("wk", [P, 8, C], BF16, isOutput=False)
    wv_d = nc.declare_dram_parameter("wv", [P, 8, C], BF16, isOutput=False)
    wo_d = nc.declare_dram_parameter("wo", [P, 4, D], BF16, isOutput=False)
    mask_d = nc.declare_dram_parameter("mask", [P, 1024], BF16, isOutput=False)
    out_d = nc.declare_dram_parameter("out", [T, D], FP32, isOutput=True)
    if debug:
        qt_dump = nc.declare_dram_parameter("qt_dump", [C, T], BF16, isOutput=True)
        kt_dump = nc.declare_dram_parameter("kt_dump", [C, T], BF16, isOutput=True)
        vp_dump = nc.declare_dram_parameter("vp_dump", [T, H * 65], BF16, isOutput=True)
        ct_dump = nc.declare_dram_parameter("ct_dump", [C, T], BF16, isOutput=True)

    Exp = mybir.ActivationFunctionType.Exp

    from contextlib import ExitStack

    with tile.TileContext(nc) as tc, ExitStack() as persist:
        const_pool = persist.enter_context(tc.tile_pool(name="const", bufs=1))
        qkt_pool = persist.enter_context(tc.tile_pool(name="qkt", bufs=1))
        vp_pool = persist.enter_context(tc.tile_pool(name="vp", bufs=1))
        persist_w = persist.enter_context(tc.tile_pool(name="pw", bufs=1))
        ctxT_pool = persist.enter_context(tc.tile_pool(name="ctxT", bufs=1))
        xt_pool = persist.enter_context(tc.tile_pool(name="xt", bufs=1))
        # PSUM: bps (projection/out-proj chains) 2x1 + st 2x2 + cs 1x2 = 8
        bps_pool = persist.enter_context(tc.tile_pool(name="bps", bufs=2, space="PSUM"))
        st_pool = persist.enter_context(tc.tile_pool(name="stps", bufs=2, space="PSUM"))
        cs_pool = persist.enter_context(tc.tile_pool(name="csps", bufs=1, space="PSUM"))
        e_pool = persist.enter_context(tc.tile_pool(name="epool", bufs=8))
        n_pool = persist.enter_context(tc.tile_pool(name="npool", bufs=2))
        o_pool = persist.enter_context(tc.tile_pool(name="osb", bufs=3))

        # ---- DMA loads: weights first, x^T chunks split over both HWDGE
        # queues (sync + scalar), t-half 0 before t-half 1 so V' and the
        # first q-spans unblock early.
        wq_sb = persist_w.tile([P, 8, C], BF16, tag="wq")
        wk_sb = persist_w.tile([P, 8, C], BF16, tag="wk")
        wv_sb = persist_w.tile([P, 8, C], BF16, tag="wv")
        wo_sb = persist_w.tile([P, 4, D], BF16, tag="wo")
        mask_sb = const_pool.tile([P, 1024], BF16, tag="mask")

        ones_sb = const_pool.tile([P, 64], FP32, tag="ones")
        nc.gpsimd.memset(ones_sb[:], 1.0)

        # Critical-path loads only (~4.5MB): wk/wq, the span-0 quarter of
        # x^T, wv+mask (SWDGE queue). The remaining 3/4 of x^T and wo are
        # deferred below so the first attention round is not starved by
        # the ~25us it takes 9MB to clear HBM.
        nc.sync.dma_start(wk_sb[:], wk_d[:])
        nc.scalar.dma_start(wq_sb[:], wq_d[:])
        nc.gpsimd.dma_start(wv_sb[:], wv_d[:])
        nc.gpsimd.dma_start(mask_sb[:], mask_d[:])

        xt = [xt_pool.tile([P, T], BF16, tag=f"xt{j}", name=f"xt{j}") for j in range(8)]
        for j in range(8):
            eng = nc.sync if j % 2 == 0 else nc.scalar
            eng.dma_start(xt[j][:, 0:SPAN], xt_d[j * P:(j + 1) * P, 0:SPAN])
        for j in range(8):
            eng = nc.sync if j % 2 == 0 else nc.scalar
            eng.dma_start(xt[j][:, SPAN:T], xt_d[j * P:(j + 1) * P, SPAN:T])

        qt = [qkt_pool.tile([P, T], BF16, tag=f"qt{i}", name=f"qt{i}") for i in range(4)]
        kt = [qkt_pool.tile([P, T], BF16, tag=f"kt{i}", name=f"kt{i}") for i in range(4)]
        vp = [vp_pool.tile([P, H, 65], BF16, tag=f"vp{t}", name=f"vp{t}") for t in range(NKB)]
        ctxT = [ctxT_pool.tile([P, T], BF16, tag=f"ct{i}", name=f"ct{i}")
                for i in range(4)]

        ci = 0

        def copy_px(dst, src, allow_scalar):
            # PSUM->SBUF evacuation; ScalarE only while it is not yet
            # exp-saturated (early rounds), VectorE otherwise.
            nonlocal ci
            ci += 1
            if allow_scalar and ci % 2 == 0:
                nc.scalar.copy(dst, src)
            else:
                nc.vector.tensor_copy(dst, src)

        def emit_qk(hp, s, allow_scalar=False):
            for dst, wsb in ((kt, wk_sb), (qt, wq_sb)):
                ps = bps_pool.tile([P, SPAN], FP32, tag="bps")
                for j in range(8):
                    nc.tensor.matmul(
                        ps[:],
                        wsb[:, j, hp * P:(hp + 1) * P],
                        xt[j][:, s * SPAN:(s + 1) * SPAN],
                        start=(j == 0), stop=(j == 7),
                    )
                copy_px(dst[hp][:, s * SPAN:(s + 1) * SPAN], ps[:], allow_scalar)

        def emit_vp(t, allow_scalar=False):
            # only the ones column needs the memset; V fills the rest
            nc.gpsimd.memset(vp[t][:, :, 64:65], 1.0)
            ps = bps_pool.tile([P, C], FP32, tag="bps")
            for j in range(8):
                nc.tensor.matmul(
                    ps[:],
                    xt[j][:, t * P:(t + 1) * P],
                    wv_sb[:, j, :],
                    start=(j == 0), stop=(j == 7),
                )
            copy_px(vp[t][:, :, 0:64],
                    ps.rearrange("p (h e) -> p h e", e=64), allow_scalar)

        def emit_C(s, hp):
            hA, hB = 2 * hp, 2 * hp + 1
            # cs: [65, 0:512] = head A ctx' (row 64 denom), [.., 512:] = B
            cs = cs_pool.tile([P, 1024], FP32, tag="cs")
            nkb = 4 * s + 4
            for kb in range(nkb):
                ksl = slice(kb * P, (kb + 1) * P)
                d = max(0, kb - 4 * s)      # diagonal offset 0..3
                q0 = s * SPAN + 128 * d     # valid q start
                w = SPAN - 128 * d          # valid width
                qsl = slice(q0, (s + 1) * SPAN)
                st = st_pool.tile([P, 1024], FP32, tag="st")
                st3 = st.rearrange("p (b q) -> p b q", b=2)[:, :, 0:w]
                nc.tensor.matmul(st[:, 0:w], kt[hp][0:64, ksl],
                                 qt[hp][0:64, qsl],
                                 start=True, stop=True)
                nc.tensor.matmul(st[:, 512:512 + w], kt[hp][64:128, ksl],
                                 qt[hp][64:128, qsl],
                                 start=True, stop=True)
                e = e_pool.tile([P, 1024], BF16, tag="e")
                e3 = e.rearrange("p (b q) -> p b q", b=2)[:, :, 0:w]
                nc.scalar.activation(e3, st3, Exp, scale=0.125)
                if d > 0 or kb == 4 * s:
                    # the staircase only affects the first 128 columns of
                    # the valid window (q0 == k0 for diagonal blocks)
                    e3m = e.rearrange("p (b q) -> p b q", b=2)[:, :, 0:128]
                    m3 = mask_sb[:, None, 384:512]
                    nc.vector.tensor_mul(
                        e3m, e3m, m3.to_broadcast((P, 2, 128)))
                co = 128 * d
                nc.tensor.matmul(cs[0:65, co:SPAN],
                                 vp[kb][:, hA, :],
                                 e[:, 0:w],
                                 start=(kb == 0), stop=(kb == nkb - 1))
                nc.tensor.matmul(cs[0:65, 512 + co:1024],
                                 vp[kb][:, hB, :],
                                 e[:, 512:512 + w],
                                 start=(kb == 0), stop=(kb == nkb - 1))
            qsl = slice(s * SPAN, (s + 1) * SPAN)
            rrb = n_pool.tile([P, 1024], FP32, tag="rrb")
            tmp = n_pool.tile([P, SPAN], BF16, tag="tmp")
            if s == 3 and hp == 3:
                # Tail: low-latency normalize. Denominator row -> SBUF,
                # broadcast to partitions 0..63 with a rank-1 matmul
                # (ones outer product; PE is idle here), reciprocal from
                # PSUM at base 0, multiply straight out of cs.
                rs = n_pool.tile([P, 1024], FP32, tag="csb")
                nc.vector.tensor_copy(rs[64:65, :], cs[64:65, :])
                psb = st_pool.tile([P, 1024], FP32, tag="st")
                for half in range(2):
                    hsl = slice(half * 512, (half + 1) * 512)
                    nc.tensor.matmul(psb[0:64, hsl], ones_sb[64:65, :],
                                     rs[64:65, hsl], start=True, stop=True)
                nc.vector.reciprocal_approx_fast(rrb[0:64, :], psb[0:64, :])
                nc.vector.tensor_mul(ctxT[hp][0:64, qsl],
                                     cs[0:64, 0:512], rrb[0:64, 0:512])
                nc.vector.tensor_mul(tmp[0:64, :],
                                     cs[0:64, 512:1024], rrb[0:64, 512:1024])
            else:
                # Evacuate ctx'+denominators to SBUF (frees the single cs
                # PSUM buffer), then normalize rows 0..63 by row 64:
                # shift-DMA the denom row to partition 0, gpsimd
                # partition_broadcast, fast reciprocal, multiply. Head A
                # lands in ctxT directly; head B shuffles to partitions
                # 64:128 by SBUF->SBUF DMA.
                csb = n_pool.tile([P, 1024], FP32, tag="csb")
                r0 = n_pool.tile([1, 1024], FP32, tag="r0")
                rsb = n_pool.tile([P, 1024], FP32, tag="rsb")
                nc.vector.tensor_copy(csb[0:65, :], cs[0:65, :])
                nc.sync.dma_start(r0[0:1, :], csb[64:65, :])
                nc.gpsimd.partition_broadcast(rsb[0:64, :], r0[0:1, :])
                nc.vector.reciprocal_approx_fast(rrb[0:64, :], rsb[0:64, :])
                nc.vector.tensor_mul(ctxT[hp][0:64, qsl],
                                     csb[0:64, 0:512], rrb[0:64, 0:512])
                nc.vector.tensor_mul(tmp[0:64, :],
                                     csb[0:64, 512:1024], rrb[0:64, 512:1024])
            nc.sync.dma_start(ctxT[hp][64:128, qsl], tmp[0:64, :])

        def emit_D_qb(qb):
            pss = [bps_pool.tile([P, SPAN], FP32, tag="bps", name=f"ops{qb}_{nh}")
                   for nh in range(2)]
            for hp in range(4):
                for nh in range(2):
                    nc.tensor.matmul(
                        pss[nh][:],
                        ctxT[hp][:, qb * P:(qb + 1) * P],
                        wo_sb[:, hp, nh * SPAN:(nh + 1) * SPAN],
                        start=(hp == 0), stop=(hp == 3),
                    )
            for nh in range(2):
                ot = o_pool.tile([P, SPAN], FP32, tag="osb")
                nc.vector.tensor_copy(ot[:], pss[nh][:])
                nc.sync.dma_start(
                    out_d[qb * P:(qb + 1) * P, nh * SPAN:(nh + 1) * SPAN],
                    ot[:])

        # ---- Interleaved rounds: one per q-span. V'/QK chains for the
        # NEXT round are prefetched inside the current round so each
        # round opens directly with attention (ScalarE never waits on a
        # projection chain), and the previous span's out-projection is
        # spread through the round to fill TensorE under exp.
        emit_qk(0, 0, allow_scalar=True)
        for t in range(4):
            emit_vp(t, allow_scalar=True)
        # wo is first needed by D(0) ~40us in; emitting its load here
        # keeps its HBM traffic out of the critical startup window
        nc.gpsimd.dma_start(wo_sb[:], wo_d[:])
        for s in range(NSPAN):
            early = s < 2
            emit_C(s, 0)
            for hp in range(1, 4):
                emit_qk(hp, s, allow_scalar=early)
            emit_C(s, 1)
            if s > 0:
                emit_D_qb(4 * (s - 1) + 0)
            if s < 3:
                emit_vp(4 * (s + 1) + 0, allow_scalar=early)
                emit_vp(4 * (s + 1) + 1, allow_scalar=early)
            emit_C(s, 2)
            if s > 0:
                emit_D_qb(4 * (s - 1) + 1)
            if s < 3:
                emit_vp(4 * (s + 1) + 2, allow_scalar=early)
                emit_vp(4 * (s + 1) + 3, allow_scalar=early)
            emit_C(s, 3)
            if s > 0:
                emit_D_qb(4 * (s - 1) + 2)
                emit_D_qb(4 * (s - 1) + 3)
            if s < 3:
                emit_qk(0, s + 1, allow_scalar=early)
        for qb in range(12, 16):
            emit_D_qb(qb)

        if debug:
            for t in range(NKB):
                nc.sync.dma_start(vp_dump[t * P:(t + 1) * P, :],
                                  vp[t].rearrange("p h e -> p (h e)"))
            for i in range(4):
                nc.sync.dma_start(qt_dump[i * P:(i + 1) * P, :], qt[i][:])
                nc.sync.dma_start(kt_dump[i * P:(i + 1) * P, :], kt[i][:])
                nc.sync.dma_start(ct_dump[i * P:(i + 1) * P, :], ctxT[i][:])

    nc.compile()
    return nc


def _get_program():
    global _program
    if _program is None:
        _program = _build()
    return _program


def _make_mask():
    import ml_dtypes
    j = np.arange(1024)[None, :]
    k = np.arange(P)[:, None]
    return np.where(j >= k + 384, 1.0, 0.0).astype(ml_dtypes.bfloat16)


def _make_in_maps(x, Wq, Wk, Wv, Wo):
    import ml_dtypes
    bf16 = ml_dtypes.bfloat16
    mask = _make_mask()
    x = np.asarray(x, np.float32)
    xts = [np.ascontiguousarray(x[b].T.astype(bf16)) for b in range(x.shape[0])]
    def chunked(w, nchunk):
        # [D_in, ncols] -> [128, nchunk, ncols]: partition p holds input
        # rows p, p+128, ... so each partition line loads contiguously
        w = np.asarray(w, np.float32).reshape(nchunk, P, -1).transpose(1, 0, 2)
        return np.ascontiguousarray(w.astype(bf16))

    in_maps = []
    for c in range(N_CORES):
        b, g = c // 2, c % 2
        cols = slice(g * C, (g + 1) * C)
        in_maps.append({
            "xt": xts[b],
            "wq": chunked(np.asarray(Wq, np.float32)[:, cols], 8),
            "wk": chunked(np.asarray(Wk, np.float32)[:, cols], 8),
            "wv": chunked(np.asarray(Wv, np.float32)[:, cols], 8),
            "wo": chunked(np.asarray(Wo, np.float32)[cols, :], 4),
            "mask": mask,
        })
    return in_maps


def _combine(results, bo, B):
    out = np.empty((B, T, D), dtype=np.float32)
    bo = np.asarray(bo, dtype=np.float32)
    for b in range(B):
        out[b] = results[2 * b]["out"] + results[2 * b + 1]["out"] + bo
    return out


def kernel(x, Wq, Wk, Wv, Wo, bo):
    x = np.asarray(x)
    nc = _get_program()
    in_maps = _make_in_maps(x, Wq, Wk, Wv, Wo)
    res = run_bass_kernel_spmd(nc, in_maps, core_ids=list(range(N_CORES)))
    return _combine(res.results, bo, x.shape[0])


def kernel_traced(x, Wq, Wk, Wv, Wo, bo):
    """Like kernel() but also returns the BassKernelResults (with
    exec_time_ns when NTFF tracing is available)."""
    x = np.asarray(x)
    nc = _get_program()
    in_maps = _make_in_maps(x, Wq, Wk, Wv, Wo)
    res = run_bass_kernel_spmd(nc, in_maps, core_ids=list(range(N_CORES)),
                               trace=True)
    return _combine(res.results, bo, x.shape[0]), res
